# revision 1
# baseline (speedup 1.0000x reference)
"""Trainium2 Bass kernel for BackprojectDepth — int8 output, u8 depth.

out[b, i, y*W+x] = depth[b,0,y,x] * (K[b,i,0]*(x+dx[b]) + K[b,i,1]*(y+dy[b]) + K[b,i,2])
out[b, 3, :] = 1.0 (host-filled).

Tolerance is 2e-2 relative to the GLOBAL output max, so the device writes
int8 with a global scale s = 127/g_ub (g_ub = exact corner-based upper
bound on max |out|, host-computed from K/dxy/depth maxima) and reads depth
as u8 (255*depth, via SWDGE cast-DMA u8->f16; the 1/255 and s factors fold
into the affine consts). Per-core HBM traffic: 2 MiB depth + 6.3 MB out +
0.3 MB consts ~= 8.6 MB (vs 16.9 MB for the fp16 baseline).

Measured HW facts driving the design (microbench2/3):
- DVE TENSOR_TENSOR with FLAT contiguous [128,N] APs runs 2x even with i8
  out (0.49 ns/el); any stride-0 broadcast / in-place / 4-dim AP drops to
  ~1x. So the depth multiply is one flat TT per (b, plane): aff[128,4096]
  * depth[128,4096] -> o8[128,4096] i8.
- DVE TENSOR_SCALAR (scale/bias f32 per-partition cols) ~0.38 ns/el; ACT
  ACTIVATE [128,1024] = 1.15 us; GP TENSOR_SCALAR ~0.9-1.2 ns/el. The 48
  affine ops (one per b,i,r: bias depends on y=4p+r) are split across
  ACT/GP/DVE per AFF_PATTERN to balance engines around the ~26 us of DVE
  TT time.
- Partition p holds rows y=4p..4p+3, so each (partition, plane) writes a
  4 KiB contiguous HBM run (best descriptor efficiency for i8).
- SWDGE (gpsimd) cast-DMA converts u8->f16 exactly on HW.
"""

import numpy as np

import concourse.bass as bass
import concourse.tile as tile
from concourse import bacc, mybir
from concourse.bass_utils import run_bass_kernel_spmd

N_CORES = 8
B, H, W = 32, 512, 1024
HW = H * W
BPC = B // N_CORES
R = H // 128  # rows per partition

F32 = mybir.dt.float32
F16 = mybir.dt.float16
I8 = mybir.dt.int8
U8 = mybir.dt.uint8

NSCBI = BPC * 3 + BPC * 3 * R  # 12 scales + 48 biases (f32)
NC = 1024 + 2 * NSCBI          # fp16 cols: xg | f32-bit-packed scbi

_TRACE = False
_LAST_RESULTS = None
_nc_cache = None

# Engine for each of the 48 affine ops, assigned round-robin from this
# string as (b, i, r) loops unroll: A=ACT, G=GpSimd, D=DVE.
DEFAULT_CFG = dict(
    aff_pattern="AGDAG" "AGAGD" "AGAGA" "GAGAG"
                "AGDAG" "AGAGD" "AGAGA" "GAGAG" "AGDAGAGA",
    depth_kicks=(1, 1, 1, 1),   # batches per SWDGE depth kick
    split_last=True,            # last batch: per-plane out kicks
)


def _build(**cfg_over):
    cfg = dict(DEFAULT_CFG, **cfg_over)
    nc = bacc.Bacc(
        "TRN2",
        target_bir_lowering=False,
        debug=False,
        enable_asserts=False,
        num_devices=N_CORES,
    )

    depth_d = nc.dram_tensor("depth", [BPC, H, W], U8, kind="ExternalInput")
    consts_d = nc.dram_tensor("consts", [128, NC], F16, kind="ExternalInput")
    out_d = nc.dram_tensor("out", [BPC, 3, HW], I8, kind="ExternalOutput")

    pat = cfg["aff_pattern"]
    assert len(pat) >= BPC * 3 * R

    with tile.TileContext(nc) as tc:
        with (
            tc.tile_pool(name="const", bufs=1) as cpool,
            tc.tile_pool(name="dpool", bufs=1) as dpool,
            tc.tile_pool(name="apool", bufs=4) as apool,
            tc.tile_pool(name="opool", bufs=3) as opool,
        ):
            ct = cpool.tile([128, NC], F16)
            nc.sync.dma_start(ct[:], consts_d.ap())
            xg = ct[:, 0:1024]
            scbi = ct[:, 1024:NC].bitcast(F32)  # [128, 60] f32

            def sc_col(b, i):
                c = 3 * b + i
                return scbi[:, c : c + 1]

            def bi_col(b, i, r):
                c = BPC * 3 + (3 * b + i) * R + r
                return scbi[:, c : c + 1]

            # depth: HBM u8 [b, (p r), m] -> SBUF f16 [p, b, r, m]
            depth_hbm = depth_d.ap().rearrange("b (p r) m -> p b r m", p=128)
            dt = dpool.tile([128, BPC, R, W], F16)
            b0 = 0
            for nb in cfg["depth_kicks"]:
                nc.gpsimd.dma_start(
                    dt[:, b0 : b0 + nb], depth_hbm[:, b0 : b0 + nb]
                )
                b0 += nb
            assert b0 == BPC

            # out: HBM i8 [b, i, ((p r m))] -> per (p,i): 4 KiB runs
            out_hbm = out_d.ap().rearrange(
                "b i (p r m) -> b p i r m", p=128, r=R
            )

            k = 0
            for b in range(BPC):
                o8 = opool.tile([128, 3, R, W], I8)
                dep_flat = dt[:, b].rearrange("p r m -> p (r m)")
                for i in range(3):
                    aff = apool.tile([128, R, W], F16)
                    for r in range(R):
                        eng = pat[k]
                        k += 1
                        if eng == "A":
                            nc.scalar.activation(
                                aff[:, r, :],
                                xg,
                                mybir.ActivationFunctionType.Identity,
                                bias=bi_col(b, i, r),
                                scale=sc_col(b, i),
                            )
                        else:
                            e = nc.vector if eng == "D" else nc.gpsimd
                            e.tensor_scalar(
                                aff[:, r, :],
                                xg,
                                sc_col(b, i),
                                bi_col(b, i, r),
                                mybir.AluOpType.mult,
                                mybir.AluOpType.add,
                            )
                    # flat 2x TT: [128, 4096] f16 * f16 -> i8
                    nc.vector.tensor_mul(
                        o8[:, i].rearrange("p r m -> p (r m)"),
                        aff[:].rearrange("p r m -> p (r m)"),
                        dep_flat,
                    )
                    if cfg["split_last"] and b == BPC - 1:
                        eng = (nc.sync, nc.scalar, nc.sync)[i]
                        eng.dma_start(out_hbm[b][:, i], o8[:, i])
                if not (cfg["split_last"] and b == BPC - 1):
                    nc.sync.dma_start(out_hbm[b], o8[:])

    nc.compile()
    return nc


def _make_in_maps(depth, inv_K, dxy):
    depth = np.asarray(depth).reshape(B, H, W)
    K = np.asarray(inv_K, dtype=np.float64)
    dxy64 = np.asarray(dxy, dtype=np.float64)

    A = K[:, :3, 0]                       # [B,3]
    Bc = K[:, :3, 1]
    C = K[:, :3, 2]
    Cc = A * dxy64[:, None, 0] + Bc * dxy64[:, None, 1] + C

    # u8 depth + per-batch max (for the exact output upper bound)
    du8 = np.rint(depth.astype(np.float64) * 255.0)
    np.clip(du8, 0.0, 255.0, out=du8)
    du8 = du8.astype(np.uint8)
    dmax = depth.max(axis=(1, 2)).astype(np.float64)  # [B]

    # exact bound: |A x + B y + Cc| maximized at grid corners
    xs = np.array([0.0, W - 1.0])
    ys = np.array([0.0, H - 1.0])
    corners = np.abs(
        A[:, :, None, None] * xs[None, None, :, None]
        + Bc[:, :, None, None] * ys[None, None, None, :]
        + Cc[:, :, None, None]
    ).max(axis=(2, 3))                    # [B,3]
    g_ub = float((corners.max(axis=1) * dmax).max())
    f = 127.0 / (g_ub * 255.0)

    p = np.arange(128, dtype=np.float64)
    in_maps = []
    for c in range(N_CORES):
        g0 = c * BPC
        consts = np.empty((128, NC), dtype=np.float16)
        consts[:, 0:1024] = np.arange(W, dtype=np.float16)[None, :]
        scbi = np.empty((128, NSCBI), dtype=np.float32)
        scbi[:, : BPC * 3] = (A[g0 : g0 + BPC] * f).reshape(BPC * 3).astype(
            np.float32
        )
        # bias[(b,i,r), p] = (B*(4p+r) + Cc) * f
        y = 4.0 * p[None, None, None, :] + np.arange(R, dtype=np.float64)[
            None, None, :, None
        ]
        bias = (
            Bc[g0 : g0 + BPC, :, None, None] * y + Cc[g0 : g0 + BPC, :, None, None]
        ) * f
        scbi[:, BPC * 3 :] = (
            bias.reshape(BPC * 3 * R, 128).T.astype(np.float32)
        )
        consts[:, 1024:NC] = scbi.view(np.float16)
        in_maps.append(
            {
                "depth": np.ascontiguousarray(du8[g0 : g0 + BPC]),
                "consts": np.ascontiguousarray(consts),
            }
        )
    return in_maps, g_ub


def _expected_inputs(nc):
    import concourse.mybir as _mybir

    names = set()
    for alloc in nc.m.functions[0].allocations:
        if (
            isinstance(alloc, _mybir.MemoryLocationSet)
            and alloc.kind == "ExternalInput"
        ):
            names.add(alloc.memorylocations[0].name)
    return names


def _run(nc, in_maps, g_ub, trace=False):
    global _LAST_RESULTS
    want = _expected_inputs(nc)
    in_maps = [{k: v for k, v in m.items() if k in want} for m in in_maps]
    res = run_bass_kernel_spmd(
        nc, in_maps, core_ids=list(range(N_CORES)), trace=trace
    )
    _LAST_RESULTS = res
    out = np.empty((B, 4, HW), dtype=np.float32)
    out[:, 3] = 1.0
    s = np.float32(g_ub / 127.0)
    for c in range(N_CORES):
        dev = res.results[c]["out"]
        out[c * BPC : (c + 1) * BPC, :3] = dev.astype(np.float32)
        out[c * BPC : (c + 1) * BPC, :3] *= s
    return out


def kernel(depth, inv_K, dxy):
    global _nc_cache
    in_maps, g_ub = _make_in_maps(depth, inv_K, dxy)
    if _nc_cache is None:
        _nc_cache = _build()
    return _run(_nc_cache, in_maps, g_ub, trace=_TRACE)


# revision 2
# speedup vs baseline: 1.1995x; 1.1995x over previous
"""Trainium2 Bass kernel for BackprojectDepth — int8 output, u8 depth.

out[b, i, y*W+x] = depth[b,0,y,x] * (K[b,i,0]*(x+dx[b]) + K[b,i,1]*(y+dy[b]) + K[b,i,2])
out[b, 3, :] = 1.0 (host-filled).

Tolerance is 2e-2 relative to the GLOBAL output max, so the device writes
int8 with a global scale s = 127/g_ub (g_ub = exact corner-based upper
bound on max |out|, host-computed from K/dxy/depth maxima) and reads depth
as u8 (255*depth, via SWDGE cast-DMA u8->f16; the 1/255 and s factors fold
into the affine consts). Per-core HBM traffic: 2 MiB depth + 6.3 MB out +
0.3 MB consts ~= 8.6 MB (vs 16.9 MB for the fp16 baseline).

Measured HW facts driving the design (microbench2/3):
- DVE TENSOR_TENSOR with FLAT contiguous [128,N] APs runs 2x even with i8
  out (0.49 ns/el); any stride-0 broadcast / in-place / 4-dim AP drops to
  ~1x. So the depth multiply is one flat TT per (b, plane): aff[128,4096]
  * depth[128,4096] -> o8[128,4096] i8.
- DVE TENSOR_SCALAR (scale/bias f32 per-partition cols) ~0.38 ns/el; ACT
  ACTIVATE [128,1024] = 1.15 us; GP TENSOR_SCALAR ~0.9-1.2 ns/el. The 48
  affine ops (one per b,i,r: bias depends on y=4p+r) are split across
  ACT/GP/DVE per AFF_PATTERN to balance engines around the ~26 us of DVE
  TT time.
- Partition p holds rows y=4p..4p+3, so each (partition, plane) writes a
  4 KiB contiguous HBM run (best descriptor efficiency for i8).
- SWDGE (gpsimd) cast-DMA converts u8->f16 exactly on HW.
"""

import numpy as np

import concourse.bass as bass
import concourse.tile as tile
from concourse import bacc, mybir
from concourse.bass_utils import run_bass_kernel_spmd

N_CORES = 8
B, H, W = 32, 512, 1024
HW = H * W
BPC = B // N_CORES
R = H // 128  # rows per partition

F32 = mybir.dt.float32
F16 = mybir.dt.float16
I8 = mybir.dt.int8
U8 = mybir.dt.uint8

NSCBI = BPC * 3 + BPC * 3 * R  # 12 scales + 48 biases (f32)
NC = 1024 + 2 * NSCBI          # fp16 cols: xg | f32-bit-packed scbi

_TRACE = False
_LAST_RESULTS = None
_nc_cache = None

# Engine for each of the 48 affine ops, assigned round-robin from this
# string as (b, i, r) loops unroll: A=ACT, G=GpSimd, D=DVE.
DEFAULT_CFG = dict(
    aff_pattern=("ADAD" + "ADAA") * 6,  # 18 DVE + 30 ACT; GP poisons SBUF
    depth_kicks=(1, 1, 1, 1),   # batches per SWDGE depth kick
    split_last=True,            # last batch: per-plane out kicks
)


def _build(**cfg_over):
    cfg = dict(DEFAULT_CFG, **cfg_over)
    nc = bacc.Bacc(
        "TRN2",
        target_bir_lowering=False,
        debug=False,
        enable_asserts=False,
        num_devices=N_CORES,
    )

    depth_d = nc.dram_tensor("depth", [BPC, H, W], U8, kind="ExternalInput")
    consts_d = nc.dram_tensor("consts", [128, NC], F16, kind="ExternalInput")
    out_d = nc.dram_tensor("out", [BPC, 3, HW], I8, kind="ExternalOutput")

    pat = cfg["aff_pattern"]
    assert len(pat) >= BPC * 3 * R

    with tile.TileContext(nc) as tc:
        with (
            tc.tile_pool(name="const", bufs=1) as cpool,
            tc.tile_pool(name="dpool", bufs=1) as dpool,
            tc.tile_pool(name="apool", bufs=4) as apool,
            tc.tile_pool(name="opool", bufs=3) as opool,
        ):
            ct = cpool.tile([128, NC], F16)
            nc.sync.dma_start(ct[:], consts_d.ap())
            xg = ct[:, 0:1024]
            scbi = ct[:, 1024:NC].bitcast(F32)  # [128, 60] f32

            def sc_col(b, i):
                c = 3 * b + i
                return scbi[:, c : c + 1]

            def bi_col(b, i, r):
                c = BPC * 3 + (3 * b + i) * R + r
                return scbi[:, c : c + 1]

            # depth: HBM u8 [b, (p r), m] -> SBUF f16 [p, b, r, m]
            depth_hbm = depth_d.ap().rearrange("b (p r) m -> p b r m", p=128)
            dt = dpool.tile([128, BPC, R, W], F16)
            b0 = 0
            for nb in cfg["depth_kicks"]:
                nc.gpsimd.dma_start(
                    dt[:, b0 : b0 + nb], depth_hbm[:, b0 : b0 + nb]
                )
                b0 += nb
            assert b0 == BPC

            # out: HBM i8 [b, i, ((p r m))] -> per (p,i): 4 KiB runs
            out_hbm = out_d.ap().rearrange(
                "b i (p r m) -> b p i r m", p=128, r=R
            )

            k = 0
            for b in range(BPC):
                o8 = opool.tile([128, 3, R, W], I8)
                dep_flat = dt[:, b].rearrange("p r m -> p (r m)")
                for i in range(3):
                    aff = apool.tile([128, R, W], F16)
                    for r in range(R):
                        eng = pat[k]
                        k += 1
                        if eng == "A":
                            nc.scalar.activation(
                                aff[:, r, :],
                                xg,
                                mybir.ActivationFunctionType.Identity,
                                bias=bi_col(b, i, r),
                                scale=sc_col(b, i),
                            )
                        else:
                            e = nc.vector if eng == "D" else nc.gpsimd
                            e.tensor_scalar(
                                aff[:, r, :],
                                xg,
                                sc_col(b, i),
                                bi_col(b, i, r),
                                mybir.AluOpType.mult,
                                mybir.AluOpType.add,
                            )
                    # flat 2x TT: [128, 4096] f16 * f16 -> i8
                    nc.vector.tensor_mul(
                        o8[:, i].rearrange("p r m -> p (r m)"),
                        aff[:].rearrange("p r m -> p (r m)"),
                        dep_flat,
                    )
                    if cfg["split_last"] and b == BPC - 1:
                        eng = (nc.sync, nc.scalar, nc.sync)[i]
                        eng.dma_start(out_hbm[b][:, i], o8[:, i])
                if not (cfg["split_last"] and b == BPC - 1):
                    nc.sync.dma_start(out_hbm[b], o8[:])

    nc.compile()
    return nc


def _make_in_maps(depth, inv_K, dxy):
    depth = np.asarray(depth).reshape(B, H, W)
    K = np.asarray(inv_K, dtype=np.float64)
    dxy64 = np.asarray(dxy, dtype=np.float64)

    A = K[:, :3, 0]                       # [B,3]
    Bc = K[:, :3, 1]
    C = K[:, :3, 2]
    Cc = A * dxy64[:, None, 0] + Bc * dxy64[:, None, 1] + C

    # u8 depth + per-batch max (for the exact output upper bound)
    du8 = np.rint(depth.astype(np.float64) * 255.0)
    np.clip(du8, 0.0, 255.0, out=du8)
    du8 = du8.astype(np.uint8)
    dmax = depth.max(axis=(1, 2)).astype(np.float64)  # [B]

    # exact bound: |A x + B y + Cc| maximized at grid corners
    xs = np.array([0.0, W - 1.0])
    ys = np.array([0.0, H - 1.0])
    corners = np.abs(
        A[:, :, None, None] * xs[None, None, :, None]
        + Bc[:, :, None, None] * ys[None, None, None, :]
        + Cc[:, :, None, None]
    ).max(axis=(2, 3))                    # [B,3]
    g_ub = float((corners.max(axis=1) * dmax).max())
    f = 127.0 / (g_ub * 255.0)

    p = np.arange(128, dtype=np.float64)
    in_maps = []
    for c in range(N_CORES):
        g0 = c * BPC
        consts = np.empty((128, NC), dtype=np.float16)
        consts[:, 0:1024] = np.arange(W, dtype=np.float16)[None, :]
        scbi = np.empty((128, NSCBI), dtype=np.float32)
        scbi[:, : BPC * 3] = (A[g0 : g0 + BPC] * f).reshape(BPC * 3).astype(
            np.float32
        )
        # bias[(b,i,r), p] = (B*(4p+r) + Cc) * f
        y = 4.0 * p[None, None, None, :] + np.arange(R, dtype=np.float64)[
            None, None, :, None
        ]
        bias = (
            Bc[g0 : g0 + BPC, :, None, None] * y + Cc[g0 : g0 + BPC, :, None, None]
        ) * f
        scbi[:, BPC * 3 :] = (
            bias.reshape(BPC * 3 * R, 128).T.astype(np.float32)
        )
        consts[:, 1024:NC] = scbi.view(np.float16)
        in_maps.append(
            {
                "depth": np.ascontiguousarray(du8[g0 : g0 + BPC]),
                "consts": np.ascontiguousarray(consts),
            }
        )
    return in_maps, g_ub


def _expected_inputs(nc):
    import concourse.mybir as _mybir

    names = set()
    for alloc in nc.m.functions[0].allocations:
        if (
            isinstance(alloc, _mybir.MemoryLocationSet)
            and alloc.kind == "ExternalInput"
        ):
            names.add(alloc.memorylocations[0].name)
    return names


def _run(nc, in_maps, g_ub, trace=False):
    global _LAST_RESULTS
    want = _expected_inputs(nc)
    in_maps = [{k: v for k, v in m.items() if k in want} for m in in_maps]
    res = run_bass_kernel_spmd(
        nc, in_maps, core_ids=list(range(N_CORES)), trace=trace
    )
    _LAST_RESULTS = res
    out = np.empty((B, 4, HW), dtype=np.float32)
    out[:, 3] = 1.0
    s = np.float32(g_ub / 127.0)
    for c in range(N_CORES):
        dev = res.results[c]["out"]
        out[c * BPC : (c + 1) * BPC, :3] = dev.astype(np.float32)
        out[c * BPC : (c + 1) * BPC, :3] *= s
    return out


def kernel(depth, inv_K, dxy):
    global _nc_cache
    in_maps, g_ub = _make_in_maps(depth, inv_K, dxy)
    if _nc_cache is None:
        _nc_cache = _build()
    return _run(_nc_cache, in_maps, g_ub, trace=_TRACE)
